# revision 3
# baseline (speedup 1.0000x reference)
"""CPO loss (top-k masking) kernel for Trainium2, 8 NeuronCores.

Problem: given logits [2, 2048, 50257] f32, target [2, 2048] int, mask
[2, 2048] f32, compute

    probs     = softmax(logits, -1)            # per row of BS = 4096 rows
    pos_prob  = probs[row, target[row]]
    neg_prob  = top-5 probs of the row
    valid     = (top5 index != target)
    row_loss  = -mean_over_valid(pos_prob - neg_prob)
    loss      = sum(row_loss * mask) / sum(mask)

Device algorithm (per core, rows sharded 512/core, 4 tiles of 128 rows):
  - stream logits chunks HBM->SBUF
  - ACT: exp(chunk) with accum_out -> per-chunk row sums (softmax denom)
  - DVE: hardware Max8 per chunk -> top-8 candidates, merged to row top-8
  - GPSIMD: indirect DMA gathers logits[row, target[row]]
  - valid mask via value inequality (top5 != t_val), which matches the
    index test because a row's values are distinct w.p. ~1; the target
    row element compares equal to itself exactly.
  - per-partition partial sums of (row_loss * mask) and mask -> out [128, 2]
Host combines the 8 cores' [128, 2] partials into the final scalar.
"""

import os

import numpy as np

import concourse.bacc as bacc
import concourse.bass as bass
import concourse.mybir as mybir
import concourse.tile as tile
from concourse.bass_utils import run_bass_kernel_spmd

B, S, V = 2, 2048, 50257
N_CORES = 8
N_ROWS = B * S                     # 4096
ROWS_PER_CORE = N_ROWS // N_CORES  # 512
P = 128
N_TILES = ROWS_PER_CORE // P       # 4
CHUNK = 8192
CHUNK_WIDTHS = [CHUNK] * (V // CHUNK) + ([V % CHUNK] if V % CHUNK else [])
N_CHUNKS = len(CHUNK_WIDTHS)       # 7 (6 x 8192 + 1105)

F32 = mybir.dt.float32
I32 = mybir.dt.int32

_compiled = {}


def _build_kernel(tc: tile.TileContext, logits: bass.AP, tgt_flat: bass.AP,
                  mask: bass.AP, out: bass.AP):
    nc = tc.nc
    ctxs = []
    stream = tc.alloc_tile_pool(name="stream", bufs=3)
    scratch_pool = tc.alloc_tile_pool(name="scratch", bufs=1)
    small = tc.alloc_tile_pool(name="small", bufs=1)

    # flat view of the logits shard for the element gather
    logits_flat = logits.rearrange("r v -> (r v)")[:, None]

    # persistent per-core small tiles
    sums_all = small.tile([P, N_TILES * N_CHUNKS], F32)   # per-chunk exp sums
    cand_all = small.tile([P, N_TILES * N_CHUNKS * 8], F32)  # per-chunk top8
    vals9 = small.tile([P, N_TILES * 9], F32)  # col t*9: t_val, t*9+1..8: top8
    tgts = small.tile([P, N_TILES], I32)
    masks = small.tile([P, N_TILES], F32)
    nc.sync.dma_start(tgts[:, :], tgt_flat)
    nc.sync.dma_start(masks[:, :], mask)

    for t in range(N_TILES):
        # gather t_val[p] = logits_flat[tgt_flat[p, t]]
        nc.gpsimd.indirect_dma_start(
            out=vals9[:, t * 9:t * 9 + 1],
            out_offset=None,
            in_=logits_flat,
            in_offset=bass.IndirectOffsetOnAxis(ap=tgts[:, t:t + 1], axis=0),
        )
        col = 0
        for c, w in enumerate(CHUNK_WIDTHS):
            chunk = stream.tile([P, CHUNK], F32, tag="chunk")
            nc.sync.dma_start(chunk[:, :w], logits[t * P:(t + 1) * P, col:col + w])
            # exp + row-sum on ACT; the exp values themselves are discarded
            escr = scratch_pool.tile([P, CHUNK], mybir.dt.bfloat16, tag="escr")
            nc.scalar.activation(
                escr[:, :w], chunk[:, :w], mybir.ActivationFunctionType.Exp,
                accum_out=sums_all[:, t * N_CHUNKS + c:t * N_CHUNKS + c + 1],
            )
            # top-8 of the raw logits chunk on DVE
            base = (t * N_CHUNKS + c) * 8
            nc.vector.max(out=cand_all[:, base:base + 8], in_=chunk[:, :w])
            col += w
        # merge the chunk candidates into the tile's top-8
        nc.vector.max(out=vals9[:, t * 9 + 1:t * 9 + 9],
                      in_=cand_all[:, t * N_CHUNKS * 8:(t + 1) * N_CHUNKS * 8])

    # ---- final small compute, vectorized over the 4 tiles ----
    v3 = vals9.rearrange("p (t n) -> p t n", n=9)
    tval = v3[:, :, 0:1]                       # [P, 4, 1]
    top5 = v3[:, :, 1:6]                       # [P, 4, 5]

    S4 = small.tile([P, N_TILES], F32)
    nc.vector.tensor_reduce(
        out=S4[:, :], in_=sums_all.rearrange("p (t c) -> p t c", c=N_CHUNKS),
        axis=mybir.AxisListType.X, op=mybir.AluOpType.add)

    valid = small.tile([P, N_TILES, 5], F32)
    nc.vector.tensor_tensor(out=valid[:], in0=top5,
                            in1=tval.to_broadcast([P, N_TILES, 5]),
                            op=mybir.AluOpType.not_equal)

    e9 = small.tile([P, N_TILES * 9], F32)
    nc.scalar.activation(e9[:, :], vals9[:, :], mybir.ActivationFunctionType.Exp)
    e3 = e9.rearrange("p (t n) -> p t n", n=9)

    diffs = small.tile([P, N_TILES, 5], F32)   # e_top5 - e_t
    nc.vector.tensor_tensor(out=diffs[:], in0=e3[:, :, 1:6],
                            in1=e3[:, :, 0:1].to_broadcast([P, N_TILES, 5]),
                            op=mybir.AluOpType.subtract)
    nc.vector.tensor_tensor(out=diffs[:], in0=diffs[:], in1=valid[:],
                            op=mybir.AluOpType.mult)

    num4 = small.tile([P, N_TILES], F32)
    nc.vector.tensor_reduce(out=num4[:, :], in_=diffs[:],
                            axis=mybir.AxisListType.X, op=mybir.AluOpType.add)
    den4 = small.tile([P, N_TILES], F32)
    nc.vector.tensor_reduce(out=den4[:, :], in_=valid[:],
                            axis=mybir.AxisListType.X, op=mybir.AluOpType.add)

    # row_loss = num / (S * den); contrib = row_loss * mask
    sd = small.tile([P, N_TILES], F32)
    nc.vector.tensor_tensor(out=sd[:, :], in0=S4[:, :], in1=den4[:, :],
                            op=mybir.AluOpType.mult)
    rec = small.tile([P, N_TILES], F32)
    nc.vector.reciprocal(out=rec[:, :], in_=sd[:, :])
    nc.vector.tensor_tensor(out=num4[:, :], in0=num4[:, :], in1=rec[:, :],
                            op=mybir.AluOpType.mult)
    nc.vector.tensor_tensor(out=num4[:, :], in0=num4[:, :], in1=masks[:, :],
                            op=mybir.AluOpType.mult)

    acc = small.tile([P, 2], F32)
    nc.vector.tensor_reduce(out=acc[:, 0:1], in_=num4[:, :],
                            axis=mybir.AxisListType.X, op=mybir.AluOpType.add)
    nc.vector.tensor_reduce(out=acc[:, 1:2], in_=masks[:, :],
                            axis=mybir.AxisListType.X, op=mybir.AluOpType.add)
    nc.sync.dma_start(out, acc[:, :])

    small.release()
    scratch_pool.release()
    stream.release()


def _build_nc() -> bass.Bass:
    nc = bacc.Bacc("TRN2", target_bir_lowering=False, debug=False,
                   enable_asserts=False, num_devices=N_CORES)
    logits = nc.dram_tensor("logits", [ROWS_PER_CORE, V], F32,
                            kind="ExternalInput")
    tgt_flat = nc.dram_tensor("tgt_flat", [P, N_TILES], I32,
                              kind="ExternalInput")
    mask = nc.dram_tensor("mask", [P, N_TILES], F32, kind="ExternalInput")
    out = nc.dram_tensor("out", [P, 2], F32, kind="ExternalOutput")
    with tile.TileContext(nc) as tc:
        _build_kernel(tc, logits.ap(), tgt_flat.ap(), mask.ap(), out.ap())
    nc.compile()
    return nc


def kernel(logits, target, mask):
    logits = np.ascontiguousarray(np.asarray(logits, dtype=np.float32))
    target = np.asarray(target).astype(np.int64)
    mask = np.asarray(mask, dtype=np.float32)

    lf = logits.reshape(N_ROWS, V)
    tf = target.reshape(N_ROWS)
    mf = mask.reshape(N_ROWS)

    in_maps = []
    for c in range(N_CORES):
        r0 = c * ROWS_PER_CORE
        rows = np.arange(ROWS_PER_CORE, dtype=np.int64)
        flat_idx = rows * V + tf[r0:r0 + ROWS_PER_CORE]
        # [p, t] layout: row of (tile t, partition p) is r0 + t*128 + p
        tgt_flat = np.ascontiguousarray(
            flat_idx.reshape(N_TILES, P).T.astype(np.int32))
        mask_pt = np.ascontiguousarray(
            mf[r0:r0 + ROWS_PER_CORE].reshape(N_TILES, P).T)
        in_maps.append({
            "logits": lf[r0:r0 + ROWS_PER_CORE],
            "tgt_flat": tgt_flat,
            "mask": mask_pt,
        })

    if "nc" not in _compiled:
        _compiled["nc"] = _build_nc()

    trace = bool(int(os.environ.get("CPO_TRACE", "0")))
    res = run_bass_kernel_spmd(
        _compiled["nc"], in_maps, core_ids=list(range(N_CORES)), trace=trace)
    kernel.last_results = res

    loss_sum = 0.0
    mask_sum = 0.0
    for r in res.results:
        o = r["out"].astype(np.float64)
        loss_sum += o[:, 0].sum()
        mask_sum += o[:, 1].sum()
    return np.float32(loss_sum / mask_sum)


# revision 10
# speedup vs baseline: 13.9993x; 13.9993x over previous
"""CPO loss (top-k masking) kernel for Trainium2, 8 NeuronCores.

Problem: given logits [2, 2048, 50257] f32, target [2, 2048] int, mask
[2, 2048] f32, compute

    probs     = softmax(logits, -1)            # per row of BS = 4096 rows
    pos_prob  = probs[row, target[row]]
    neg_prob  = top-5 probs of the row
    valid     = (top5 index != target)
    row_loss  = -mean_over_valid(pos_prob - neg_prob)
    loss      = sum(row_loss * mask) / sum(mask)

Device algorithm (per core, rows sharded 512/core, 4 tiles of 128 rows):
  - stream logits chunks HBM->SBUF
  - ACT: exp(chunk) with accum_out -> per-chunk row sums (softmax denom)
  - DVE: hardware Max8 per chunk -> top-8 candidates, merged to row top-8
  - GPSIMD: indirect DMA gathers logits[row, target[row]]
  - valid mask via value inequality (top5 != t_val), which matches the
    index test because a row's values are distinct w.p. ~1; the target
    row element compares equal to itself exactly.
  - per-partition partial sums of (row_loss * mask) and mask -> out [128, 2]
Host combines the 8 cores' [128, 2] partials into the final scalar.
"""

import numpy as np

import concourse.bacc as bacc
import concourse.bass as bass
import concourse.mybir as mybir
import concourse.tile as tile
from concourse.bass_utils import run_bass_kernel_spmd

B, S, V = 2, 2048, 50257
N_CORES = 8
N_ROWS = B * S                     # 4096
ROWS_PER_CORE = N_ROWS // N_CORES  # 512
P = 128
N_TILES = ROWS_PER_CORE // P       # 4
CHUNK = 16384
CHUNK_WIDTHS = [CHUNK] * (V // CHUNK) + ([V % CHUNK] if V % CHUNK else [])
N_CHUNKS = len(CHUNK_WIDTHS)       # 4 (3 x 16384 + 1105)

F32 = mybir.dt.float32
I32 = mybir.dt.int32

_compiled = {}


SPLIT_MAX = False   # offload first-level max pooling to GPSIMD


def _build_kernel(tc: tile.TileContext, logits: bass.AP, tgt_flat: bass.AP,
                  mask: bass.AP, out: bass.AP):
    nc = tc.nc
    stream = tc.alloc_tile_pool(name="stream", bufs=2)
    scratch_pool = tc.alloc_tile_pool(name="scratch", bufs=1)
    small = tc.alloc_tile_pool(name="small", bufs=1)

    # flat view of the logits shard for the element gather
    logits_flat = logits.rearrange("r v -> (r v)")[:, None]

    # persistent per-core small tiles
    sums_all = small.tile([P, N_TILES * N_CHUNKS], F32)   # per-chunk exp sums
    cand_all = small.tile([P, N_TILES * N_CHUNKS * 8], F32)  # per-chunk top8
    vals9 = small.tile([P, N_TILES * 9], F32)  # col t*9: t_val, t*9+1..8: top8
    tgts = small.tile([P, N_TILES], I32)
    masks = small.tile([P, N_TILES], F32)
    nc.sync.dma_start(tgts[:, :], tgt_flat)
    nc.sync.dma_start(masks[:, :], mask)

    for t in range(N_TILES):
        # gather t_val[p] = logits_flat[tgt_flat[p, t]]
        nc.gpsimd.indirect_dma_start(
            out=vals9[:, t * 9:t * 9 + 1],
            out_offset=None,
            in_=logits_flat,
            in_offset=bass.IndirectOffsetOnAxis(ap=tgts[:, t:t + 1], axis=0),
        )
        col = 0
        for c, w in enumerate(CHUNK_WIDTHS):
            chunk = stream.tile([P, CHUNK], F32, tag="chunk")
            nc.sync.dma_start(chunk[:, :w], logits[t * P:(t + 1) * P, col:col + w])
            # exp + row-sum on ACT; the exp values themselves are discarded
            escr = scratch_pool.tile([P, CHUNK], mybir.dt.bfloat16, tag="escr")
            nc.scalar.activation(
                escr[:, :w], chunk[:, :w], mybir.ActivationFunctionType.Exp,
                accum_out=sums_all[:, t * N_CHUNKS + c:t * N_CHUNKS + c + 1],
            )
            # top-8 of the raw logits chunk on DVE; optionally let GPSIMD
            # pre-pool pairs (elementwise max of the chunk halves) so the
            # DVE Max8 only scans half the elements
            base = (t * N_CHUNKS + c) * 8
            if SPLIT_MAX and w == CHUNK:
                half = CHUNK // 2
                pooled = scratch_pool.tile([P, half], F32, tag="pooled",
                                           bufs=2)
                nc.gpsimd.tensor_tensor(out=pooled[:, :],
                                        in0=chunk[:, 0:half],
                                        in1=chunk[:, half:CHUNK],
                                        op=mybir.AluOpType.max)
                nc.vector.max(out=cand_all[:, base:base + 8],
                              in_=pooled[:, :])
            else:
                nc.vector.max(out=cand_all[:, base:base + 8],
                              in_=chunk[:, :w])
            col += w
        # merge the chunk candidates into the tile's top-8
        nc.vector.max(out=vals9[:, t * 9 + 1:t * 9 + 9],
                      in_=cand_all[:, t * N_CHUNKS * 8:(t + 1) * N_CHUNKS * 8])

    # ---- final small compute, vectorized over the 4 tiles ----
    v3 = vals9.rearrange("p (t n) -> p t n", n=9)
    tval = v3[:, :, 0:1]                       # [P, 4, 1]
    top5 = v3[:, :, 1:6]                       # [P, 4, 5]

    S4 = small.tile([P, N_TILES], F32)
    nc.vector.tensor_reduce(
        out=S4[:, :], in_=sums_all.rearrange("p (t c) -> p t c", c=N_CHUNKS),
        axis=mybir.AxisListType.X, op=mybir.AluOpType.add)

    valid = small.tile([P, N_TILES, 5], F32)
    nc.vector.tensor_tensor(out=valid[:], in0=top5,
                            in1=tval.to_broadcast([P, N_TILES, 5]),
                            op=mybir.AluOpType.not_equal)

    e9 = small.tile([P, N_TILES * 9], F32)
    nc.scalar.activation(e9[:, :], vals9[:, :], mybir.ActivationFunctionType.Exp)
    e3 = e9.rearrange("p (t n) -> p t n", n=9)

    diffs = small.tile([P, N_TILES, 5], F32)   # e_top5 - e_t
    nc.vector.tensor_tensor(out=diffs[:], in0=e3[:, :, 1:6],
                            in1=e3[:, :, 0:1].to_broadcast([P, N_TILES, 5]),
                            op=mybir.AluOpType.subtract)
    nc.vector.tensor_tensor(out=diffs[:], in0=diffs[:], in1=valid[:],
                            op=mybir.AluOpType.mult)

    num4 = small.tile([P, N_TILES], F32)
    nc.vector.tensor_reduce(out=num4[:, :], in_=diffs[:],
                            axis=mybir.AxisListType.X, op=mybir.AluOpType.add)
    den4 = small.tile([P, N_TILES], F32)
    nc.vector.tensor_reduce(out=den4[:, :], in_=valid[:],
                            axis=mybir.AxisListType.X, op=mybir.AluOpType.add)

    # row_loss = num / (S * den); contrib = row_loss * mask
    sd = small.tile([P, N_TILES], F32)
    nc.vector.tensor_tensor(out=sd[:, :], in0=S4[:, :], in1=den4[:, :],
                            op=mybir.AluOpType.mult)
    rec = small.tile([P, N_TILES], F32)
    nc.vector.reciprocal(out=rec[:, :], in_=sd[:, :])
    nc.vector.tensor_tensor(out=num4[:, :], in0=num4[:, :], in1=rec[:, :],
                            op=mybir.AluOpType.mult)
    nc.vector.tensor_tensor(out=num4[:, :], in0=num4[:, :], in1=masks[:, :],
                            op=mybir.AluOpType.mult)

    acc = small.tile([P, 2], F32)
    nc.vector.tensor_reduce(out=acc[:, 0:1], in_=num4[:, :],
                            axis=mybir.AxisListType.X, op=mybir.AluOpType.add)
    nc.vector.tensor_reduce(out=acc[:, 1:2], in_=masks[:, :],
                            axis=mybir.AxisListType.X, op=mybir.AluOpType.add)
    nc.sync.dma_start(out, acc[:, :])

    small.release()
    scratch_pool.release()
    stream.release()


def _build_nc(reps: int = 1) -> bass.Bass:
    """reps > 1 repeats the whole body; used only for timing calibration."""
    nc = bacc.Bacc("TRN2", target_bir_lowering=False, debug=False,
                   enable_asserts=False, num_devices=N_CORES)
    logits = nc.dram_tensor("logits", [ROWS_PER_CORE, V], F32,
                            kind="ExternalInput")
    tgt_flat = nc.dram_tensor("tgt_flat", [P, N_TILES], I32,
                              kind="ExternalInput")
    mask = nc.dram_tensor("mask", [P, N_TILES], F32, kind="ExternalInput")
    out = nc.dram_tensor("out", [P, 2], F32, kind="ExternalOutput")
    with tile.TileContext(nc) as tc:
        for _ in range(reps):
            _build_kernel(tc, logits.ap(), tgt_flat.ap(), mask.ap(), out.ap())
    nc.compile()
    return nc


def kernel(logits, target, mask):
    logits = np.ascontiguousarray(np.asarray(logits, dtype=np.float32))
    target = np.asarray(target).astype(np.int64)
    mask = np.asarray(mask, dtype=np.float32)

    lf = logits.reshape(N_ROWS, V)
    tf = target.reshape(N_ROWS)
    mf = mask.reshape(N_ROWS)

    in_maps = []
    for c in range(N_CORES):
        r0 = c * ROWS_PER_CORE
        rows = np.arange(ROWS_PER_CORE, dtype=np.int64)
        flat_idx = rows * V + tf[r0:r0 + ROWS_PER_CORE]
        # [p, t] layout: row of (tile t, partition p) is r0 + t*128 + p
        tgt_flat = np.ascontiguousarray(
            flat_idx.reshape(N_TILES, P).T.astype(np.int32))
        mask_pt = np.ascontiguousarray(
            mf[r0:r0 + ROWS_PER_CORE].reshape(N_TILES, P).T)
        in_maps.append({
            "logits": lf[r0:r0 + ROWS_PER_CORE],
            "tgt_flat": tgt_flat,
            "mask": mask_pt,
        })

    if "nc" not in _compiled:
        _compiled["nc"] = _build_nc()

    # NTFF tracing is unavailable under this axon client (antenv.axon_hooks
    # is absent), so always run trace-free.
    res = run_bass_kernel_spmd(
        _compiled["nc"], in_maps, core_ids=list(range(N_CORES)), trace=False)
    kernel.last_results = res

    loss_sum = 0.0
    mask_sum = 0.0
    for r in res.results:
        o = r["out"].astype(np.float64)
        loss_sum += o[:, 0].sum()
        mask_sum += o[:, 1].sum()
    return np.float32(loss_sum / mask_sum)


# revision 11
# speedup vs baseline: 25.1079x; 1.7935x over previous
"""CPO loss (top-k masking) kernel for Trainium2, 8 NeuronCores.

Problem: given logits [2, 2048, 50257] f32, target [2, 2048] int, mask
[2, 2048] f32, compute

    probs     = softmax(logits, -1)            # per row of BS = 4096 rows
    pos_prob  = probs[row, target[row]]
    neg_prob  = top-5 probs of the row
    valid     = (top5 index != target)
    row_loss  = -mean_over_valid(pos_prob - neg_prob)
    loss      = sum(row_loss * mask) / sum(mask)

Device algorithm (per core, rows sharded 512/core, 4 tiles of 128 rows):
  - stream logits chunks HBM->SBUF
  - ACT: exp(chunk) with accum_out -> per-chunk row sums (softmax denom)
  - DVE: hardware Max8 per chunk -> top-8 candidates, merged to row top-8
  - GPSIMD: indirect DMA gathers logits[row, target[row]]
  - valid mask via value inequality (top5 != t_val), which matches the
    index test because a row's values are distinct w.p. ~1; the target
    row element compares equal to itself exactly.
  - per-partition partial sums of (row_loss * mask) and mask -> out [128, 2]
Host combines the 8 cores' [128, 2] partials into the final scalar.
"""

import numpy as np

import concourse.bacc as bacc
import concourse.bass as bass
import concourse.mybir as mybir
import concourse.tile as tile
from concourse.bass_utils import run_bass_kernel_spmd

B, S, V = 2, 2048, 50257
N_CORES = 8
N_ROWS = B * S                     # 4096
ROWS_PER_CORE = N_ROWS // N_CORES  # 512
P = 128
N_TILES = ROWS_PER_CORE // P       # 4
CHUNK = 8192
CHUNK_WIDTHS = [CHUNK] * (V // CHUNK) + ([V % CHUNK] if V % CHUNK else [])
N_CHUNKS = len(CHUNK_WIDTHS)       # 7 (6 x 8192 + 1105)

F32 = mybir.dt.float32
I32 = mybir.dt.int32

_compiled = {}


SPLIT_MAX = False   # offload first-level max pooling to GPSIMD


def _build_kernel(tc: tile.TileContext, logits: bass.AP, tgt_flat: bass.AP,
                  mask: bass.AP, out: bass.AP):
    nc = tc.nc
    stream = tc.alloc_tile_pool(name="stream", bufs=3)
    scratch_pool = tc.alloc_tile_pool(name="scratch", bufs=1)
    small = tc.alloc_tile_pool(name="small", bufs=1)

    # flat view of the logits shard for the element gather
    logits_flat = logits.rearrange("r v -> (r v)")[:, None]

    # persistent per-core small tiles
    sums_all = small.tile([P, N_TILES * N_CHUNKS], F32)   # per-chunk exp sums
    cand_all = small.tile([P, N_TILES * N_CHUNKS * 8], F32)  # per-chunk top8
    vals9 = small.tile([P, N_TILES * 9], F32)  # col t*9: t_val, t*9+1..8: top8
    tgts = small.tile([P, N_TILES], I32)
    masks = small.tile([P, N_TILES], F32)
    nc.sync.dma_start(tgts[:, :], tgt_flat)
    nc.sync.dma_start(masks[:, :], mask)

    for t in range(N_TILES):
        # gather t_val[p] = logits_flat[tgt_flat[p, t]]
        nc.gpsimd.indirect_dma_start(
            out=vals9[:, t * 9:t * 9 + 1],
            out_offset=None,
            in_=logits_flat,
            in_offset=bass.IndirectOffsetOnAxis(ap=tgts[:, t:t + 1], axis=0),
        )
        col = 0
        for c, w in enumerate(CHUNK_WIDTHS):
            chunk = stream.tile([P, CHUNK], F32, tag="chunk")
            nc.sync.dma_start(chunk[:, :w], logits[t * P:(t + 1) * P, col:col + w])
            # exp + row-sum on ACT; the exp values themselves are discarded
            escr = scratch_pool.tile([P, CHUNK], mybir.dt.bfloat16, tag="escr")
            nc.scalar.activation(
                escr[:, :w], chunk[:, :w], mybir.ActivationFunctionType.Exp,
                accum_out=sums_all[:, t * N_CHUNKS + c:t * N_CHUNKS + c + 1],
            )
            # top-8 of the raw logits chunk on DVE; optionally let GPSIMD
            # pre-pool pairs (elementwise max of the chunk halves) so the
            # DVE Max8 only scans half the elements
            base = (t * N_CHUNKS + c) * 8
            if SPLIT_MAX and w == CHUNK:
                half = CHUNK // 2
                pooled = scratch_pool.tile([P, half], F32, tag="pooled",
                                           bufs=2)
                nc.gpsimd.tensor_tensor(out=pooled[:, :],
                                        in0=chunk[:, 0:half],
                                        in1=chunk[:, half:CHUNK],
                                        op=mybir.AluOpType.max)
                nc.vector.max(out=cand_all[:, base:base + 8],
                              in_=pooled[:, :])
            else:
                nc.vector.max(out=cand_all[:, base:base + 8],
                              in_=chunk[:, :w])
            col += w
        # merge the chunk candidates into the tile's top-8
        nc.vector.max(out=vals9[:, t * 9 + 1:t * 9 + 9],
                      in_=cand_all[:, t * N_CHUNKS * 8:(t + 1) * N_CHUNKS * 8])

    # ---- final small compute, vectorized over the 4 tiles ----
    v3 = vals9.rearrange("p (t n) -> p t n", n=9)
    tval = v3[:, :, 0:1]                       # [P, 4, 1]
    top5 = v3[:, :, 1:6]                       # [P, 4, 5]

    S4 = small.tile([P, N_TILES], F32)
    nc.vector.tensor_reduce(
        out=S4[:, :], in_=sums_all.rearrange("p (t c) -> p t c", c=N_CHUNKS),
        axis=mybir.AxisListType.X, op=mybir.AluOpType.add)

    valid = small.tile([P, N_TILES, 5], F32)
    nc.vector.tensor_tensor(out=valid[:], in0=top5,
                            in1=tval.to_broadcast([P, N_TILES, 5]),
                            op=mybir.AluOpType.not_equal)

    e9 = small.tile([P, N_TILES * 9], F32)
    nc.scalar.activation(e9[:, :], vals9[:, :], mybir.ActivationFunctionType.Exp)
    e3 = e9.rearrange("p (t n) -> p t n", n=9)

    diffs = small.tile([P, N_TILES, 5], F32)   # e_top5 - e_t
    nc.vector.tensor_tensor(out=diffs[:], in0=e3[:, :, 1:6],
                            in1=e3[:, :, 0:1].to_broadcast([P, N_TILES, 5]),
                            op=mybir.AluOpType.subtract)
    nc.vector.tensor_tensor(out=diffs[:], in0=diffs[:], in1=valid[:],
                            op=mybir.AluOpType.mult)

    num4 = small.tile([P, N_TILES], F32)
    nc.vector.tensor_reduce(out=num4[:, :], in_=diffs[:],
                            axis=mybir.AxisListType.X, op=mybir.AluOpType.add)
    den4 = small.tile([P, N_TILES], F32)
    nc.vector.tensor_reduce(out=den4[:, :], in_=valid[:],
                            axis=mybir.AxisListType.X, op=mybir.AluOpType.add)

    # row_loss = num / (S * den); contrib = row_loss * mask
    sd = small.tile([P, N_TILES], F32)
    nc.vector.tensor_tensor(out=sd[:, :], in0=S4[:, :], in1=den4[:, :],
                            op=mybir.AluOpType.mult)
    rec = small.tile([P, N_TILES], F32)
    nc.vector.reciprocal(out=rec[:, :], in_=sd[:, :])
    nc.vector.tensor_tensor(out=num4[:, :], in0=num4[:, :], in1=rec[:, :],
                            op=mybir.AluOpType.mult)
    nc.vector.tensor_tensor(out=num4[:, :], in0=num4[:, :], in1=masks[:, :],
                            op=mybir.AluOpType.mult)

    acc = small.tile([P, 2], F32)
    nc.vector.tensor_reduce(out=acc[:, 0:1], in_=num4[:, :],
                            axis=mybir.AxisListType.X, op=mybir.AluOpType.add)
    nc.vector.tensor_reduce(out=acc[:, 1:2], in_=masks[:, :],
                            axis=mybir.AxisListType.X, op=mybir.AluOpType.add)
    nc.sync.dma_start(out, acc[:, :])

    small.release()
    scratch_pool.release()
    stream.release()


def _build_nc(reps: int = 1) -> bass.Bass:
    """reps > 1 repeats the whole body; used only for timing calibration."""
    nc = bacc.Bacc("TRN2", target_bir_lowering=False, debug=False,
                   enable_asserts=False, num_devices=N_CORES)
    logits = nc.dram_tensor("logits", [ROWS_PER_CORE, V], F32,
                            kind="ExternalInput")
    tgt_flat = nc.dram_tensor("tgt_flat", [P, N_TILES], I32,
                              kind="ExternalInput")
    mask = nc.dram_tensor("mask", [P, N_TILES], F32, kind="ExternalInput")
    out = nc.dram_tensor("out", [P, 2], F32, kind="ExternalOutput")
    with tile.TileContext(nc) as tc:
        for _ in range(reps):
            _build_kernel(tc, logits.ap(), tgt_flat.ap(), mask.ap(), out.ap())
    nc.compile()
    return nc


def kernel(logits, target, mask):
    logits = np.ascontiguousarray(np.asarray(logits, dtype=np.float32))
    target = np.asarray(target).astype(np.int64)
    mask = np.asarray(mask, dtype=np.float32)

    lf = logits.reshape(N_ROWS, V)
    tf = target.reshape(N_ROWS)
    mf = mask.reshape(N_ROWS)

    in_maps = []
    for c in range(N_CORES):
        r0 = c * ROWS_PER_CORE
        rows = np.arange(ROWS_PER_CORE, dtype=np.int64)
        flat_idx = rows * V + tf[r0:r0 + ROWS_PER_CORE]
        # [p, t] layout: row of (tile t, partition p) is r0 + t*128 + p
        tgt_flat = np.ascontiguousarray(
            flat_idx.reshape(N_TILES, P).T.astype(np.int32))
        mask_pt = np.ascontiguousarray(
            mf[r0:r0 + ROWS_PER_CORE].reshape(N_TILES, P).T)
        in_maps.append({
            "logits": lf[r0:r0 + ROWS_PER_CORE],
            "tgt_flat": tgt_flat,
            "mask": mask_pt,
        })

    if "nc" not in _compiled:
        _compiled["nc"] = _build_nc()

    # NTFF tracing is unavailable under this axon client (antenv.axon_hooks
    # is absent), so always run trace-free.
    res = run_bass_kernel_spmd(
        _compiled["nc"], in_maps, core_ids=list(range(N_CORES)), trace=False)
    kernel.last_results = res

    loss_sum = 0.0
    mask_sum = 0.0
    for r in res.results:
        o = r["out"].astype(np.float64)
        loss_sum += o[:, 0].sum()
        mask_sum += o[:, 1].sum()
    return np.float32(loss_sum / mask_sum)
